# revision 19
# baseline (speedup 1.0000x reference)
"""Trainium2 Bass kernel for nn_AttnDecoderRNN: 50-step LSTM decoder with
argmax feedback over a 32000 vocab, distributed over 8 NeuronCores.

Sharding: vocab-parallel output projection (4000/core, out_w resident in SBUF
as bf16 hi/lo), H-sharded LSTM (128 hidden units/core), embedding replicated
(indirect-DMA gather by token). Per step: AllGather of the h^T slice (bf16
hi/lo) and AllGather of local argmax candidates.

Numerics: all matmuls are bf16 hi/lo 3-pass ("bf16x3": hi@Whi + lo@Whi +
hi@Wlo, fp32 PSUM accumulate), which reproduces fp32 logits to ~1e-6 --
needed because the min top-2 logit gap is ~1.2e-4 and one argmax flip
cascades through the whole trajectory.

Layout notes:
 - batch B=64 is doubled onto 128 partitions: partition p = (sub, b) with
   sub = p//64 selecting one half of the core's 128 hidden units (for the
   LSTM cell) or one of two 512-wide vocab chunks (for logits). This feeds
   the PE col-tiling trick: two concurrent matmuls at tile_position (0,0)
   and (0,64) fill all 128 array columns despite M=64.
 - Gate weights are host-permuted to [sub, gate, j] column order; std is
   host-permuted to [t, (sub,b), (chunk,col)] and out_b is folded in.
"""
import os
import sys
import time

sys.path.insert(0, "/opt/trn_rl_repo/concourse")
sys.path.insert(0, "/opt/trn_rl_repo")

import numpy as np
import ml_dtypes

import concourse.bass as bass
import concourse.mybir as mybir
import concourse.tile as tile
from concourse import bacc
from concourse.bass_utils import run_bass_kernel_spmd
from concourse import bass2jax as _b2j
import jax
from jax.experimental.shard_map import shard_map
from jax.sharding import Mesh, NamedSharding, PartitionSpec

dt = mybir.dt
bf16 = ml_dtypes.bfloat16

B = 64            # batch
H = 1024          # hidden
V = 32000         # vocab
T = int(os.environ.get("KERNEL_T", "50"))  # steps
C = 8             # cores
VS = V // C       # vocab slice per core (4000)
HS = H // C       # hidden slice per core (128)
KT_Z = 2 * H // 128   # 16 z^T K-tiles (x: 0..7, h: 8..15)
KT_H = H // 128       # 8 h^T K-tiles
NPAIR = 4             # col-tiled vocab pairs
CS = VS // (2 * NPAIR)  # 500 cols per half-chunk
BIG = float(1 << 15)  # idx < 32000 < 2^15; BIG - idx stays exactly representable
SOS = 1

SCHEME = os.environ.get("KERNEL_SCHEME", "bf16x3")  # bf16x3 | f32r
SIG_VIA_TANH = os.environ.get("KERNEL_SIG_TANH", "0") == "1"
NO_CC = os.environ.get("KERNEL_NO_CC", "0") == "1"  # timing diagnostic only

F = mybir.ActivationFunctionType
A = mybir.AluOpType

_cache = {}


def _passes():
    """(lhs_sel, rhs_sel) pairs: which zT / W copy each matmul pass uses."""
    if SCHEME == "bf16x3":
        return [("hi", "hi"), ("lo", "hi"), ("hi", "lo")]
    return [("r", "r")]


def build():
    nc = bacc.Bacc("TRN2", target_bir_lowering=False, debug=False,
                   num_devices=C)
    wdt = dt.bfloat16 if SCHEME == "bf16x3" else dt.float32r

    # ---- inputs (per core) ----
    def inp(name, shape, dtp):
        return nc.dram_tensor(name, shape, dtp, kind="ExternalInput")

    if SCHEME == "bf16x3":
        i_embc = inp("emb_cat", [V, 2 * H], wdt)
        emb_offs = [("hi", 0), ("lo", H)]
    else:
        i_embc = inp("emb_cat", [V, H], wdt)
        emb_offs = [("r", 0)]
    i_wz = {s: inp(f"wz_{s}", [KT_Z, 128, 512], wdt) for s in set(
        x for p in _passes() for x in p)}
    i_wv = {s: inp(f"wv_{s}", [KT_H, 128, VS], wdt) for s in set(
        x for p in _passes() for x in p)}
    i_h0 = {s: inp(f"h0T_{s}", [KT_H, 128, 64], wdt) for s in set(
        x for p in _passes() for x in p)}
    i_bias = inp("bias", [128, 256], dt.float32)
    i_c0 = inp("c0p", [128, 64], dt.float32)
    i_coff = inp("cand_off", [128, 4], dt.float32)
    i_std = inp("stdp", [T, 128, NPAIR * CS], dt.float32)
    i_ident = inp("ident", [128, 128], dt.float32)
    i_identb = inp("identb", [128, 128], dt.bfloat16)

    # ---- outputs ----
    o_toks = nc.dram_tensor("toks_out", [64, T], dt.int32, kind="ExternalOutput")
    o_h = nc.dram_tensor("h_out", [128, 64], dt.float32, kind="ExternalOutput")

    # ---- internal DRAM for collectives (double buffered) ----
    xw = 128 if SCHEME == "bf16x3" else 64   # exchange width (hi|lo or single)
    xdt = dt.bfloat16 if SCHEME == "bf16x3" else dt.float32
    agh_in = [nc.dram_tensor(f"agh_in{i}", [64, 2 * xw], xdt, kind="Internal")
              for i in range(2)]
    agh_out = [nc.dram_tensor(f"agh_out{i}", [C * 64, 2 * xw], xdt,
                              kind="Internal", addr_space="Shared")
               for i in range(2)]
    agc_in = [nc.dram_tensor(f"agc_in{i}", [128, 8], dt.float32,
                             kind="Internal") for i in range(2)]
    agc_out = [nc.dram_tensor(f"agc_out{i}", [C * 128, 8], dt.float32,
                              kind="Internal", addr_space="Shared")
               for i in range(2)]
    rg = [list(range(C))]

    with tile.TileContext(nc) as tc:
        with tc.tile_pool(name="persist", bufs=1) as pp, \
             tc.tile_pool(name="work", bufs=2) as wp, \
             tc.tile_pool(name="gath", bufs=1) as gp, \
             tc.tile_pool(name="pmm", bufs=2, space="PSUM") as pmm, \
             tc.tile_pool(name="pg", bufs=1, space="PSUM") as pgp:

            # ---- resident tiles ----
            t_wz = {s: pp.tile([128, KT_Z * 512], wdt, tag=f"wz{s}", name=f"t_wz_{s}")
                    for s in i_wz}
            t_wv = {s: pp.tile([128, KT_H * VS], wdt, tag=f"wv{s}", name=f"t_wv_{s}")
                    for s in i_wv}
            t_bias = pp.tile([128, 256], dt.float32, tag="bias")
            t_coff = pp.tile([128, 4], dt.float32, tag="coff")
            t_ident = pp.tile([128, 128], dt.float32, tag="ident")
            t_identb = pp.tile([128, 128], dt.bfloat16, tag="identb")
            # z^T K-tiles: cols [k*64:(k+1)*64] for k in 0..15
            t_zT = {s: pp.tile([128, KT_Z * 64], wdt, tag=f"zT{s}", name=f"t_zT_{s}", uniquify=False)
                    for s in i_wz}
            t_c = pp.tile([128, 64], dt.float32, tag="c_st", name="t_c", uniquify=False)
            t_h = pp.tile([128, 64], dt.float32, tag="h_sl", name="t_h", uniquify=False)
            t_tokf = pp.tile([64, 1], dt.float32, tag="tokf")
            t_toku = pp.tile([64, 1], dt.uint32, tag="toku")
            t_hist = pp.tile([64, T], dt.int32, tag="hist")

            for s in t_wz:
                for k in range(KT_Z):
                    nc.sync.dma_start(t_wz[s][:, k*512:(k+1)*512], i_wz[s].ap()[k])
            for s in t_wv:
                for k in range(KT_H):
                    nc.sync.dma_start(t_wv[s][:, k*VS:(k+1)*VS], i_wv[s].ap()[k])
            nc.sync.dma_start(t_bias[:], i_bias.ap())
            nc.sync.dma_start(t_coff[:], i_coff.ap())
            nc.sync.dma_start(t_ident[:], i_ident.ap())
            nc.sync.dma_start(t_identb[:], i_identb.ap())
            nc.sync.dma_start(t_c[:], i_c0.ap())
            # initial h^T (k-tiles 8..15 of zT)
            for s in t_zT:
                key = s if s in i_h0 else "hi"
                for k in range(KT_H):
                    nc.sync.dma_start(t_zT[s][:, (KT_H + k)*64:(KT_H + k + 1)*64],
                                      i_h0[key].ap()[k])
            nc.gpsimd.memset(t_toku[:], SOS)

            zsel = {"hi": t_zT.get("hi"), "lo": t_zT.get("lo"),
                    "r": t_zT.get("r")}

            def zt(s, k):
                return zsel[s][:, k*64:(k+1)*64]

            for t in range(T):
                par = t % 2

                # prefetch this step's std early (double-buffered)
                std_t = gp.tile([128, NPAIR * CS], dt.float32, tag="std",
                                bufs=2, name=f"std_{t}", uniquify=False)
                nc.sync.dma_start(std_t[:], i_std.ap()[t])

                # ---- A: gather x = emb[tok] (hi|lo in one row), transpose
                x_raw = gp.tile([64, i_embc.shape[1]], wdt, tag="xraw",
                                name=f"x_raw_{t}")
                nc.gpsimd.indirect_dma_start(
                    out=x_raw[:], out_offset=None, in_=i_embc.ap(),
                    in_offset=bass.IndirectOffsetOnAxis(
                        ap=t_toku[:, :1], axis=0))
                isbf = wdt == dt.bfloat16
                tid = t_identb if isbf else t_ident
                for k in range(KT_H):
                    for s, off in emb_offs:
                        ps = pmm.tile([128, 64],
                                      dt.bfloat16 if isbf else dt.float32,
                                      tag="tr", name=f"ps_xtr_{s}_{t}_{k}")
                        nc.tensor.matmul(ps[:],
                                         x_raw[:, off + k*128:off + (k+1)*128],
                                         tid[:64, :64], is_transpose=True,
                                         start=True, stop=True)
                        nc.vector.tensor_copy(zt(s, k), ps[:])

                # ---- B: gates matmul (h-part K-tiles first: they're ready
                # before tok arrives, so PE can run them during the cand-AG)
                psGA = pgp.tile([128, 256], dt.float32, tag="gA")
                psGB = pgp.tile([128, 256], dt.float32, tag="gB")
                korder = list(range(KT_H, KT_Z)) + list(range(KT_H))
                pss = _passes()
                n_mm = len(pss) * len(korder)
                i_mm = 0
                for ls, rs in pss:
                    for k in korder:
                        first = i_mm == 0
                        last = i_mm == n_mm - 1
                        nc.tensor.matmul(
                            psGA[:64, :], zt(ls, k),
                            t_wz[rs][:, k*512:k*512 + 256],
                            start=first, stop=last, tile_position=(0, 0),
                            skip_group_check=True)
                        nc.tensor.matmul(
                            psGB[64:128, :], zt(ls, k),
                            t_wz[rs][:, k*512 + 256:(k+1)*512],
                            start=first, stop=last, tile_position=(0, 64),
                            skip_group_check=True)
                        i_mm += 1

                # ---- C: cell update on [128, 64] (p = (sub, b)) ----
                g_sb = wp.tile([128, 256], dt.float32, tag="gsb", name=f"g_sb_{t}", uniquify=False)
                nc.vector.tensor_tensor(g_sb[:64, :], psGA[:64, :],
                                        t_bias[:64, :], op=A.add)
                nc.vector.tensor_tensor(g_sb[64:, :], psGB[64:128, :],
                                        t_bias[64:, :], op=A.add)
                sig_if = wp.tile([128, 128], dt.float32, tag="sigif")
                tanh_g = wp.tile([128, 64], dt.float32, tag="tanhg")
                sig_o = wp.tile([128, 64], dt.float32, tag="sigo")
                nc.scalar.activation(sig_if[:], g_sb[:, 0:128], F.Sigmoid)
                nc.scalar.activation(sig_o[:], g_sb[:, 192:256], F.Sigmoid)
                nc.scalar.activation(tanh_g[:], g_sb[:, 128:192], F.Tanh)
                tmp1 = wp.tile([128, 64], dt.float32, tag="tmp1")
                tmp2 = wp.tile([128, 64], dt.float32, tag="tmp2")
                nc.vector.tensor_tensor(tmp1[:], sig_if[:, 64:128], t_c[:],
                                        op=A.mult)
                nc.vector.tensor_tensor(tmp2[:], sig_if[:, 0:64], tanh_g[:],
                                        op=A.mult)
                nc.vector.tensor_tensor(t_c[:], tmp1[:], tmp2[:], op=A.add)
                tanh_c = wp.tile([128, 64], dt.float32, tag="tanhc")
                nc.scalar.activation(tanh_c[:], t_c[:], F.Tanh)
                nc.vector.tensor_tensor(t_h[:], sig_o[:], tanh_c[:], op=A.mult)

                # ---- D: h^T slice via 2 PE transposes, split, AllGather ----
                h_slB = wp.tile([64, 64], dt.float32, tag="hslB")
                nc.sync.dma_start(h_slB[:], t_h[64:128, :])
                psTa = pmm.tile([64, 64], dt.float32, tag="tr")
                psTb = pmm.tile([64, 64], dt.float32, tag="tr")
                nc.tensor.matmul(psTa[:], t_h[:64, :], t_ident[:64, :64],
                                 is_transpose=True, start=True, stop=True)
                nc.tensor.matmul(psTb[:], h_slB[:], t_ident[:64, :64],
                                 is_transpose=True, start=True, stop=True)
                xch = wp.tile([64, 2 * xw], xdt, tag="xch")
                nc.vector.tensor_copy(xch[:, 0:64], psTa[:])
                nc.vector.tensor_copy(xch[:, 64:128], psTb[:])
                tmpT = wp.tile([64, 64], dt.float32, tag="tmpT")
                nc.vector.tensor_tensor(tmpT[:], psTa[:], xch[:, 0:64],
                                        op=A.subtract)
                nc.vector.tensor_copy(xch[:, 128:192], tmpT[:])
                nc.vector.tensor_tensor(tmpT[:], psTb[:], xch[:, 64:128],
                                        op=A.subtract)
                nc.vector.tensor_copy(xch[:, 192:256], tmpT[:])
                nc.sync.dma_start(agh_in[par].ap(), xch[:])
                if not NO_CC:
                    nc.gpsimd.collective_compute(
                        "AllGather", A.bypass, replica_groups=rg,
                        ins=[agh_in[par].ap()], outs=[agh_out[par].ap()])
                # keep the PE HAM clock-gate open during the AllGather wait:
                # ~2us of junk matmuls so the idle stretch stays < the 3.4us
                # MID window (else the projection starts at 1.2 GHz each step)
                warm = pmm.tile([128, 512], dt.float32, tag="vA",
                                name=f"warm_{t}")
                for wi in range(25):
                    nc.tensor.matmul(warm[:64, :64], t_identb[:, :64],
                                     t_identb[:, 64:128], start=True,
                                     stop=True)
                # agh_out row 64k+j holds [hiA|hiB|loA|loB] of core k, hT row j
                src = agh_out[par].ap().rearrange("(k j) c -> j k c", k=C)
                hpart_hi = t_zT["hi"][:, KT_H*64:].rearrange(
                    "p (k b) -> p k b", k=C)
                hpart_lo = t_zT["lo"][:, KT_H*64:].rearrange(
                    "p (k b) -> p k b", k=C)
                nc.sync.dma_start(hpart_hi[0:64], src[:, :, 0:64])
                nc.sync.dma_start(hpart_hi[64:128], src[:, :, 64:128])
                nc.sync.dma_start(hpart_lo[0:64], src[:, :, 128:192])
                nc.sync.dma_start(hpart_lo[64:128], src[:, :, 192:256])

                # ---- E: output projection + local top-1 per 512-chunk ----
                mx8 = wp.tile([128, 32], dt.float32, tag="mx8", name=f"mx8_{t}", uniquify=False)
                mi8 = wp.tile([128, 32], dt.uint32, tag="mi8", name=f"mi8_{t}", uniquify=False)
                for j in range(NPAIR):
                    psA = pmm.tile([128, 512], dt.float32, tag="vA")
                    psB = pmm.tile([128, 512], dt.float32, tag="vB")
                    i_mm = 0
                    n_mm2 = len(pss) * KT_H
                    for ls, rs in pss:
                        for k in range(KT_H):
                            first = i_mm == 0
                            last = i_mm == n_mm2 - 1
                            nc.tensor.matmul(
                                psA[:64, :CS], zt(ls, KT_H + k),
                                t_wv[rs][:, k*VS + 2*CS*j:k*VS + 2*CS*j + CS],
                                start=first, stop=last, tile_position=(0, 0),
                                skip_group_check=True)
                            nc.tensor.matmul(
                                psB[64:128, :CS], zt(ls, KT_H + k),
                                t_wv[rs][:, k*VS + 2*CS*j + CS:
                                         k*VS + 2*CS*(j+1)],
                                start=first, stop=last, tile_position=(0, 64),
                                skip_group_check=True)
                            i_mm += 1
                    lg = wp.tile([128, CS], dt.float32, tag="lg", name=f"lg_{t}_{j}", uniquify=False)
                    nc.vector.tensor_tensor(lg[:64, :], psA[:64, :CS],
                                            std_t[:64, j*CS:(j+1)*CS],
                                            op=A.add)
                    nc.vector.tensor_tensor(lg[64:, :], psB[64:128, :CS],
                                            std_t[64:, j*CS:(j+1)*CS],
                                            op=A.add)
                    nc.vector.max(mx8[:, j*8:(j+1)*8], lg[:])
                    nc.vector.max_index(mi8[:, j*8:(j+1)*8],
                                        mx8[:, j*8:(j+1)*8], lg[:])

                # ---- F: pack candidates, AllGather, combine ----
                xchC = wp.tile([128, 8], dt.float32, tag="xchC")
                mxv = mx8[:].rearrange("p (j e) -> p j e", e=8)[:, :, 0]
                miv = mi8[:].rearrange("p (j e) -> p j e", e=8)[:, :, 0]
                candf = wp.tile([128, 4], dt.float32, tag="candf")
                nc.vector.tensor_copy(xchC[:, 0:4], mxv)
                nc.vector.tensor_copy(candf[:], miv)
                nc.vector.tensor_tensor(xchC[:, 4:8], candf[:], t_coff[:],
                                        op=A.add)
                nc.sync.dma_start(agc_in[par].ap(), xchC[:])
                if not NO_CC:
                    nc.gpsimd.collective_compute(
                        "AllGather", A.bypass, replica_groups=rg,
                        ins=[agc_in[par].ap()], outs=[agc_out[par].ap()])
                vals = wp.tile([64, 64], dt.float32, tag="vals", name=f"vals_{t}", uniquify=False)
                idxs = wp.tile([64, 64], dt.float32, tag="idxs", name=f"idxs_{t}", uniquify=False)
                srcc = agc_out[par].ap().rearrange(
                    "(co half b) x -> b co half x", co=C, half=2)
                nc.sync.dma_start(
                    vals[:].rearrange("b (co half j) -> b co half j",
                                      co=C, half=2), srcc[:, :, :, 0:4])
                nc.sync.dma_start(
                    idxs[:].rearrange("b (co half j) -> b co half j",
                                      co=C, half=2), srcc[:, :, :, 4:8])
                m1 = wp.tile([64, 1], dt.float32, tag="m1")
                nc.vector.reduce_max(m1[:], vals[:], axis=mybir.AxisListType.X)
                eqm = wp.tile([64, 64], dt.float32, tag="eqm")
                nc.vector.tensor_scalar(eqm[:], vals[:], m1[:, :1], None,
                                        op0=A.is_equal)
                nidx = wp.tile([64, 64], dt.float32, tag="nidx")
                nc.vector.tensor_scalar(nidx[:], idxs[:], -1.0, BIG,
                                        op0=A.mult, op1=A.add)
                key = wp.tile([64, 64], dt.float32, tag="key")
                nc.vector.tensor_tensor(key[:], eqm[:], nidx[:], op=A.mult)
                sel = wp.tile([64, 1], dt.float32, tag="sel")
                nc.vector.reduce_max(sel[:], key[:], axis=mybir.AxisListType.X)
                nc.vector.tensor_scalar(t_tokf[:], sel[:], -1.0, BIG,
                                        op0=A.mult, op1=A.add)
                nc.vector.tensor_copy(t_toku[:], t_tokf[:])
                nc.vector.tensor_copy(t_hist[:, t:t+1], t_tokf[:])

            # ---- final outputs ----
            nc.sync.dma_start(o_toks.ap(), t_hist[:])
            nc.sync.dma_start(o_h.ap(), t_h[:])

    nc.compile()
    return nc


def _split(x):
    hi = x.astype(bf16)
    lo = (x - hi.astype(np.float32)).astype(bf16)
    return hi, lo


def _prep_inputs(encoder_output, std, h, c, emb, w_ih, w_hh, b_ih, b_hh,
                 out_w, out_b):
    """Build the 8 per-core in_maps (all numpy, host side)."""
    emb = np.asarray(emb, np.float32)
    std = np.asarray(std, np.float32)
    h = np.asarray(h, np.float32)
    c = np.asarray(c, np.float32)
    w_ih = np.asarray(w_ih, np.float32)
    w_hh = np.asarray(w_hh, np.float32)
    bias = np.asarray(b_ih, np.float32) + np.asarray(b_hh, np.float32)  # [4,H]
    out_w = np.asarray(out_w, np.float32)
    out_b = np.asarray(out_b, np.float32)

    # z weights: [2048, 4, 1024] (K = [x;h], gate, H_out)
    Wz = np.concatenate([w_ih, w_hh], axis=1).transpose(1, 0, 2)
    ident = np.eye(128, dtype=np.float32)

    if SCHEME == "bf16x3":
        ehi, elo = _split(emb)
        emb_cat = np.concatenate([ehi, elo], axis=1)
        h0T_full = {k: v for k, v in zip(("hi", "lo"), _split(h.T.copy()))}
    else:
        emb_cat = emb
        h0T_full = {"r": h.T.copy()}

    in_maps = []
    for cc in range(C):
        m = {"emb_cat": emb_cat}
        # gate columns permuted to [sub(2), gate(4), j(64)] for col-tiling
        wzc = Wz[:, :, HS*cc:HS*(cc+1)]                  # [2048, 4, 128]
        wzc = wzc.reshape(2048, 4, 2, 64).transpose(0, 2, 1, 3) \
                 .reshape(2048, 512)
        wvc = out_w[:, VS*cc:VS*(cc+1)]                  # [1024, 4000]
        if SCHEME == "bf16x3":
            for s, v in zip(("hi", "lo"), _split(wzc)):
                m[f"wz_{s}"] = v.reshape(KT_Z, 128, 512)
            for s, v in zip(("hi", "lo"), _split(wvc)):
                m[f"wv_{s}"] = v.reshape(KT_H, 128, VS)
            for s in ("hi", "lo"):
                m[f"h0T_{s}"] = h0T_full[s].reshape(KT_H, 128, 64)
        else:
            m["wz_r"] = wzc.reshape(KT_Z, 128, 512)
            m["wv_r"] = wvc.reshape(KT_H, 128, VS)
            m["h0T_r"] = h0T_full["r"].reshape(KT_H, 128, 64)
        bc = bias[:, HS*cc:HS*(cc+1)].reshape(4, 2, 64).transpose(1, 0, 2) \
                 .reshape(2, 256)                        # [sub, gate*64]
        m["bias"] = np.repeat(bc, 64, axis=0).astype(np.float32)  # [128,256]
        m["c0p"] = np.ascontiguousarray(
            c[:, HS*cc:HS*(cc+1)].reshape(64, 2, 64).transpose(1, 0, 2)
            .reshape(128, 64), np.float32)
        p = np.arange(128)
        j = np.arange(4)
        m["cand_off"] = (VS*cc + 2*CS*j[None, :]
                         + CS*(p[:, None] // 64)).astype(np.float32)
        # std: [T, 64, 4000] -> [T, 128, 2048] with col j*512+q ->
        # vocab VS*cc + 1024*j + 512*sub + q, plus out_b folded in
        sc = std[:T, :, VS*cc:VS*(cc+1)] + out_b[None, None, VS*cc:VS*(cc+1)]
        sc = sc.reshape(T, 64, NPAIR, 2, CS).transpose(0, 3, 1, 2, 4) \
               .reshape(T, 128, NPAIR * CS)
        m["stdp"] = np.ascontiguousarray(sc, np.float32)
        m["ident"] = ident
        m["identb"] = ident.astype(bf16)
        in_maps.append(m)
    return in_maps


def _run_pjrt(nc, in_maps, time_iters=0):
    """Mirror of bass2jax.run_bass_via_pjrt with pre-placed device inputs and
    an optional timing loop (wall time of execute with inputs resident)."""
    import concourse.mybir as mb
    _b2j.install_neuronx_cc_hook()
    if nc.dbg_addr is not None:
        in_maps = [{**m, nc.dbg_addr.name: np.zeros((1, 2), np.uint32)}
                   for m in in_maps]
    partition_name = (nc.partition_id_tensor.name
                      if nc.partition_id_tensor else None)
    in_names, out_names, out_avals, zero_outs = [], [], [], []
    for alloc in nc.m.functions[0].allocations:
        if not isinstance(alloc, mb.MemoryLocationSet):
            continue
        name = alloc.memorylocations[0].name
        if alloc.kind == "ExternalInput":
            if name != partition_name:
                in_names.append(name)
        elif alloc.kind == "ExternalOutput":
            out_names.append(name)
            shape = tuple(alloc.tensor_shape)
            dtp = mb.dt.np(alloc.dtype)
            out_avals.append(jax.core.ShapedArray(shape, dtp))
            zero_outs.append(np.zeros(shape, dtp))
    n_params = len(in_names)
    n_outs = len(out_avals)
    in_names.extend(out_names)
    if partition_name is not None:
        in_names.append(partition_name)

    def _body(*args):
        operands = list(args)
        if partition_name is not None:
            operands.append(_b2j.partition_id_tensor())
        outs = _b2j._bass_exec_p.bind(
            *operands, out_avals=tuple(out_avals), in_names=tuple(in_names),
            out_names=tuple(out_names), lowering_input_output_aliases=(),
            sim_require_finite=True, sim_require_nnan=True, nc=nc)
        return tuple(outs)

    devices = jax.devices()[:C]
    mesh = Mesh(np.asarray(devices), ("core",))
    in_specs = (PartitionSpec("core"),) * (n_params + n_outs)
    out_specs = (PartitionSpec("core"),) * len(out_names)
    sharded = jax.jit(shard_map(_body, mesh=mesh, in_specs=in_specs,
                                out_specs=out_specs, check_rep=False),
                      keep_unused=True)
    sh = NamedSharding(mesh, PartitionSpec("core"))
    concat_in = [
        jax.device_put(np.concatenate(
            [np.asarray(in_maps[c][in_names[i]]) for c in range(C)], axis=0),
            sh)
        for i in range(n_params)
    ]
    concat_zeros = [
        jax.device_put(np.zeros((C * z.shape[0], *z.shape[1:]), z.dtype), sh)
        for z in zero_outs
    ]
    t0 = time.time()
    out_arrs = jax.block_until_ready(sharded(*concat_in, *concat_zeros))
    print(f"[kernel] first exec (incl compile): {time.time()-t0:.1f}s",
          file=sys.stderr)
    exec_times = []
    for _ in range(time_iters):
        t0 = time.time()
        out_arrs2 = jax.block_until_ready(sharded(*concat_in, *concat_zeros))
        exec_times.append(time.time() - t0)
        del out_arrs2
    results = [
        {name: np.asarray(out_arrs[i]).reshape(C, *out_avals[i].shape)[c]
         for i, name in enumerate(out_names)}
        for c in range(C)
    ]
    return results, (min(exec_times) if exec_times else None)


def kernel(**inputs):
    if "nc" not in _cache:
        t0 = time.time()
        _cache["nc"] = build()
        print(f"[kernel] build: {time.time()-t0:.1f}s", file=sys.stderr)
    nc = _cache["nc"]
    in_maps = _prep_inputs(**inputs)
    t0 = time.time()
    results, best_s = _run_pjrt(nc, in_maps,
                                time_iters=int(os.environ.get("KERNEL_TIME_ITERS", "0")))
    print(f"[kernel] run: {time.time()-t0:.1f}s", file=sys.stderr)
    _cache["exec_wall_s"] = best_s

    toks = results[0]["toks_out"].T.astype(np.int32)  # [T, 64]
    h_full = np.zeros((64, H), np.float32)
    for cc in range(C):
        hv = results[cc]["h_out"].reshape(2, 64, 64)     # [sub, b, j]
        h_full[:, HS*cc:HS*cc+64] = hv[0]
        h_full[:, HS*cc+64:HS*(cc+1)] = hv[1]
    return toks, h_full


# revision 20
# speedup vs baseline: 1.2954x; 1.2954x over previous
"""Trainium2 Bass kernel for nn_AttnDecoderRNN: 50-step LSTM decoder with
argmax feedback over a 32000 vocab, distributed over 8 NeuronCores.

Sharding: vocab-parallel output projection (4000/core, out_w resident in SBUF
as bf16 hi/lo), H-sharded LSTM (128 hidden units/core), embedding replicated
(indirect-DMA gather by token). Per step: AllGather of the h^T slice (bf16
hi/lo) and AllGather of local argmax candidates.

Numerics: all matmuls are bf16 hi/lo 3-pass ("bf16x3": hi@Whi + lo@Whi +
hi@Wlo, fp32 PSUM accumulate), which reproduces fp32 logits to ~1e-6 --
needed because the min top-2 logit gap is ~1.2e-4 and one argmax flip
cascades through the whole trajectory.

Layout notes:
 - batch B=64 is doubled onto 128 partitions: partition p = (sub, b) with
   sub = p//64 selecting one half of the core's 128 hidden units (for the
   LSTM cell) or one of two 512-wide vocab chunks (for logits). This feeds
   the PE col-tiling trick: two concurrent matmuls at tile_position (0,0)
   and (0,64) fill all 128 array columns despite M=64.
 - Gate weights are host-permuted to [sub, gate, j] column order; std is
   host-permuted to [t, (sub,b), (chunk,col)] and out_b is folded in.
"""
import os
import sys
import time

sys.path.insert(0, "/opt/trn_rl_repo/concourse")
sys.path.insert(0, "/opt/trn_rl_repo")

import numpy as np
import ml_dtypes

import concourse.bass as bass
import concourse.mybir as mybir
import concourse.tile as tile
from concourse import bacc
from concourse.bass_utils import run_bass_kernel_spmd
from concourse import bass2jax as _b2j
import jax
from jax.experimental.shard_map import shard_map
from jax.sharding import Mesh, NamedSharding, PartitionSpec

dt = mybir.dt
bf16 = ml_dtypes.bfloat16

B = 64            # batch
H = 1024          # hidden
V = 32000         # vocab
T = int(os.environ.get("KERNEL_T", "50"))  # steps
C = 8             # cores
VS = V // C       # vocab slice per core (4000)
HS = H // C       # hidden slice per core (128)
KT_Z = 2 * H // 128   # 16 z^T K-tiles (x: 0..7, h: 8..15)
KT_H = H // 128       # 8 h^T K-tiles
NPAIR = 4             # col-tiled vocab pairs
CS = VS // (2 * NPAIR)  # 500 cols per half-chunk
BIG = float(1 << 15)  # idx < 32000 < 2^15; BIG - idx stays exactly representable
SOS = 1

SCHEME = os.environ.get("KERNEL_SCHEME", "bf16x3")  # bf16x3 | f32r
SIG_VIA_TANH = os.environ.get("KERNEL_SIG_TANH", "0") == "1"
NO_CC = os.environ.get("KERNEL_NO_CC", "0") == "1"  # timing diagnostic only

F = mybir.ActivationFunctionType
A = mybir.AluOpType

_cache = {}


def _passes():
    """(lhs_sel, rhs_sel) pairs: which zT / W copy each matmul pass uses."""
    if SCHEME == "bf16x3":
        return [("hi", "hi"), ("lo", "hi"), ("hi", "lo")]
    return [("r", "r")]


def build():
    nc = bacc.Bacc("TRN2", target_bir_lowering=False, debug=False,
                   num_devices=C)
    wdt = dt.bfloat16 if SCHEME == "bf16x3" else dt.float32r

    # ---- inputs (per core) ----
    def inp(name, shape, dtp):
        return nc.dram_tensor(name, shape, dtp, kind="ExternalInput")

    if SCHEME == "bf16x3":
        i_embc = inp("emb_cat", [V, 2 * H], wdt)
        emb_offs = [("hi", 0), ("lo", H)]
    else:
        i_embc = inp("emb_cat", [V, H], wdt)
        emb_offs = [("r", 0)]
    i_wz = {s: inp(f"wz_{s}", [KT_Z, 128, 512], wdt) for s in set(
        x for p in _passes() for x in p)}
    i_wv = {s: inp(f"wv_{s}", [KT_H, 128, VS], wdt) for s in set(
        x for p in _passes() for x in p)}
    i_h0 = {s: inp(f"h0T_{s}", [KT_H, 128, 64], wdt) for s in set(
        x for p in _passes() for x in p)}
    i_bias = inp("bias", [128, 256], dt.float32)
    i_c0 = inp("c0p", [128, 64], dt.float32)
    i_coff = inp("cand_off", [128, 4], dt.float32)
    i_std = inp("stdp", [T, 128, NPAIR * CS], dt.float32)
    i_ident = inp("ident", [128, 128], dt.float32)
    i_identb = inp("identb", [128, 128], dt.bfloat16)

    # ---- outputs ----
    o_toks = nc.dram_tensor("toks_out", [64, T], dt.int32, kind="ExternalOutput")
    o_h = nc.dram_tensor("h_out", [128, 64], dt.float32, kind="ExternalOutput")

    # ---- internal DRAM for collectives (double buffered) ----
    xw = 128 if SCHEME == "bf16x3" else 64   # exchange width (hi|lo or single)
    xdt = dt.bfloat16 if SCHEME == "bf16x3" else dt.float32
    agh_in = [nc.dram_tensor(f"agh_in{i}", [64, 2 * xw], xdt, kind="Internal")
              for i in range(2)]
    agh_out = [nc.dram_tensor(f"agh_out{i}", [C * 64, 2 * xw], xdt,
                              kind="Internal", addr_space="Shared")
               for i in range(2)]
    agc_in = [nc.dram_tensor(f"agc_in{i}", [128, 8], dt.float32,
                             kind="Internal") for i in range(2)]
    agc_out = [nc.dram_tensor(f"agc_out{i}", [C * 128, 8], dt.float32,
                              kind="Internal", addr_space="Shared")
               for i in range(2)]
    rg = [list(range(C))]

    with tile.TileContext(nc) as tc:
        with tc.tile_pool(name="persist", bufs=1) as pp, \
             tc.tile_pool(name="work", bufs=2) as wp, \
             tc.tile_pool(name="gath", bufs=1) as gp, \
             tc.tile_pool(name="pmm", bufs=2, space="PSUM") as pmm, \
             tc.tile_pool(name="pg", bufs=1, space="PSUM") as pgp:

            # ---- resident tiles ----
            t_wz = {s: pp.tile([128, KT_Z * 512], wdt, tag=f"wz{s}", name=f"t_wz_{s}")
                    for s in i_wz}
            t_wv = {s: pp.tile([128, KT_H * VS], wdt, tag=f"wv{s}", name=f"t_wv_{s}")
                    for s in i_wv}
            t_bias = pp.tile([128, 256], dt.float32, tag="bias")
            t_coff = pp.tile([128, 4], dt.float32, tag="coff")
            t_ident = pp.tile([128, 128], dt.float32, tag="ident")
            t_identb = pp.tile([128, 128], dt.bfloat16, tag="identb")
            # z^T K-tiles: cols [k*64:(k+1)*64] for k in 0..15
            t_zT = {s: pp.tile([128, KT_Z * 64], wdt, tag=f"zT{s}", name=f"t_zT_{s}", uniquify=False)
                    for s in i_wz}
            t_c = pp.tile([128, 64], dt.float32, tag="c_st", name="t_c", uniquify=False)
            t_h = pp.tile([128, 64], dt.float32, tag="h_sl", name="t_h", uniquify=False)
            t_tokf = pp.tile([64, 1], dt.float32, tag="tokf")
            t_toku = pp.tile([64, 1], dt.uint32, tag="toku")
            t_hist = pp.tile([64, T], dt.int32, tag="hist")

            for s in t_wz:
                for k in range(KT_Z):
                    nc.sync.dma_start(t_wz[s][:, k*512:(k+1)*512], i_wz[s].ap()[k])
            for s in t_wv:
                for k in range(KT_H):
                    nc.sync.dma_start(t_wv[s][:, k*VS:(k+1)*VS], i_wv[s].ap()[k])
            nc.sync.dma_start(t_bias[:], i_bias.ap())
            nc.sync.dma_start(t_coff[:], i_coff.ap())
            nc.sync.dma_start(t_ident[:], i_ident.ap())
            nc.sync.dma_start(t_identb[:], i_identb.ap())
            nc.sync.dma_start(t_c[:], i_c0.ap())
            # initial h^T (k-tiles 8..15 of zT)
            for s in t_zT:
                key = s if s in i_h0 else "hi"
                for k in range(KT_H):
                    nc.sync.dma_start(t_zT[s][:, (KT_H + k)*64:(KT_H + k + 1)*64],
                                      i_h0[key].ap()[k])
            nc.gpsimd.memset(t_toku[:], SOS)

            zsel = {"hi": t_zT.get("hi"), "lo": t_zT.get("lo"),
                    "r": t_zT.get("r")}

            def zt(s, k):
                return zsel[s][:, k*64:(k+1)*64]

            for t in range(T):
                par = t % 2

                # ---- A: gather x = emb[tok] (hi|lo in one row), transpose
                x_raw = gp.tile([64, i_embc.shape[1]], wdt, tag="xraw",
                                name=f"x_raw_{t}")
                nc.gpsimd.indirect_dma_start(
                    out=x_raw[:], out_offset=None, in_=i_embc.ap(),
                    in_offset=bass.IndirectOffsetOnAxis(
                        ap=t_toku[:, :1], axis=0))
                isbf = wdt == dt.bfloat16
                tid = t_identb if isbf else t_ident
                for k in range(KT_H):
                    for s, off in emb_offs:
                        ps = pmm.tile([128, 64],
                                      dt.bfloat16 if isbf else dt.float32,
                                      tag="tr", name=f"ps_xtr_{s}_{t}_{k}")
                        nc.tensor.matmul(ps[:],
                                         x_raw[:, off + k*128:off + (k+1)*128],
                                         tid[:64, :64], is_transpose=True,
                                         start=True, stop=True)
                        nc.vector.tensor_copy(zt(s, k), ps[:])

                # ---- B: gates matmul (h-part K-tiles first: they're ready
                # before tok arrives, so PE can run them during the cand-AG)
                psGA = pgp.tile([128, 256], dt.float32, tag="gA")
                psGB = pgp.tile([128, 256], dt.float32, tag="gB")
                korder = list(range(KT_H, KT_Z)) + list(range(KT_H))
                pss = _passes()
                n_mm = len(pss) * len(korder)
                i_mm = 0
                for ls, rs in pss:
                    for k in korder:
                        first = i_mm == 0
                        last = i_mm == n_mm - 1
                        nc.tensor.matmul(
                            psGA[:64, :], zt(ls, k),
                            t_wz[rs][:, k*512:k*512 + 256],
                            start=first, stop=last, tile_position=(0, 0),
                            skip_group_check=True)
                        nc.tensor.matmul(
                            psGB[64:128, :], zt(ls, k),
                            t_wz[rs][:, k*512 + 256:(k+1)*512],
                            start=first, stop=last, tile_position=(0, 64),
                            skip_group_check=True)
                        i_mm += 1

                # ---- C: cell update on [128, 64] (p = (sub, b)) ----
                g_sb = wp.tile([128, 256], dt.float32, tag="gsb", name=f"g_sb_{t}", uniquify=False)
                nc.vector.tensor_tensor(g_sb[:64, :], psGA[:64, :],
                                        t_bias[:64, :], op=A.add)
                nc.vector.tensor_tensor(g_sb[64:, :], psGB[64:128, :],
                                        t_bias[64:, :], op=A.add)
                sig_if = wp.tile([128, 128], dt.float32, tag="sigif")
                tanh_g = wp.tile([128, 64], dt.float32, tag="tanhg")
                sig_o = wp.tile([128, 64], dt.float32, tag="sigo")
                nc.scalar.activation(sig_if[:], g_sb[:, 0:128], F.Sigmoid)
                nc.scalar.activation(sig_o[:], g_sb[:, 192:256], F.Sigmoid)
                nc.scalar.activation(tanh_g[:], g_sb[:, 128:192], F.Tanh)
                tmp1 = wp.tile([128, 64], dt.float32, tag="tmp1")
                tmp2 = wp.tile([128, 64], dt.float32, tag="tmp2")
                nc.vector.tensor_tensor(tmp1[:], sig_if[:, 64:128], t_c[:],
                                        op=A.mult)
                nc.vector.tensor_tensor(tmp2[:], sig_if[:, 0:64], tanh_g[:],
                                        op=A.mult)
                nc.vector.tensor_tensor(t_c[:], tmp1[:], tmp2[:], op=A.add)
                tanh_c = wp.tile([128, 64], dt.float32, tag="tanhc")
                nc.scalar.activation(tanh_c[:], t_c[:], F.Tanh)
                nc.vector.tensor_tensor(t_h[:], sig_o[:], tanh_c[:], op=A.mult)

                # ---- D: h^T slice via 2 PE transposes, split, AllGather ----
                h_slB = wp.tile([64, 64], dt.float32, tag="hslB")
                nc.sync.dma_start(h_slB[:], t_h[64:128, :])
                psTa = pmm.tile([64, 64], dt.float32, tag="tr")
                psTb = pmm.tile([64, 64], dt.float32, tag="tr")
                nc.tensor.matmul(psTa[:], t_h[:64, :], t_ident[:64, :64],
                                 is_transpose=True, start=True, stop=True)
                nc.tensor.matmul(psTb[:], h_slB[:], t_ident[:64, :64],
                                 is_transpose=True, start=True, stop=True)
                xch = wp.tile([64, 2 * xw], xdt, tag="xch")
                nc.vector.tensor_copy(xch[:, 0:64], psTa[:])
                nc.vector.tensor_copy(xch[:, 64:128], psTb[:])
                tmpT = wp.tile([64, 64], dt.float32, tag="tmpT")
                nc.vector.tensor_tensor(tmpT[:], psTa[:], xch[:, 0:64],
                                        op=A.subtract)
                nc.vector.tensor_copy(xch[:, 128:192], tmpT[:])
                nc.vector.tensor_tensor(tmpT[:], psTb[:], xch[:, 64:128],
                                        op=A.subtract)
                nc.vector.tensor_copy(xch[:, 192:256], tmpT[:])
                nc.sync.dma_start(agh_in[par].ap(), xch[:])
                if not NO_CC:
                    nc.gpsimd.collective_compute(
                        "AllGather", A.bypass, replica_groups=rg,
                        ins=[agh_in[par].ap()], outs=[agh_out[par].ap()])
                # keep the PE HAM clock-gate open during the AllGather wait:
                # ~2us of junk matmuls so the idle stretch stays < the 3.4us
                # MID window (else the projection starts at 1.2 GHz each step)
                warm = pmm.tile([128, 512], dt.float32, tag="vA",
                                name=f"warm_{t}")
                for wi in range(25):
                    nc.tensor.matmul(warm[:64, :64], t_identb[:, :64],
                                     t_identb[:, 64:128], start=True,
                                     stop=True)
                # agh_out row 64k+j holds [hiA|hiB|loA|loB] of core k, hT row j
                src = agh_out[par].ap().rearrange("(k j) c -> j k c", k=C)
                hpart_hi = t_zT["hi"][:, KT_H*64:].rearrange(
                    "p (k b) -> p k b", k=C)
                hpart_lo = t_zT["lo"][:, KT_H*64:].rearrange(
                    "p (k b) -> p k b", k=C)
                nc.sync.dma_start(hpart_hi[0:64], src[:, :, 0:64])
                nc.sync.dma_start(hpart_hi[64:128], src[:, :, 64:128])
                nc.sync.dma_start(hpart_lo[0:64], src[:, :, 128:192])
                nc.sync.dma_start(hpart_lo[64:128], src[:, :, 192:256])

                # ---- E: output projection + local top-1 per 512-chunk ----
                std_t = gp.tile([128, NPAIR * CS], dt.float32, tag="std")
                nc.sync.dma_start(std_t[:], i_std.ap()[t])
                mx8 = wp.tile([128, 32], dt.float32, tag="mx8", name=f"mx8_{t}", uniquify=False)
                mi8 = wp.tile([128, 32], dt.uint32, tag="mi8", name=f"mi8_{t}", uniquify=False)
                for j in range(NPAIR):
                    psA = pmm.tile([128, 512], dt.float32, tag="vA")
                    psB = pmm.tile([128, 512], dt.float32, tag="vB")
                    i_mm = 0
                    n_mm2 = len(pss) * KT_H
                    for ls, rs in pss:
                        for k in range(KT_H):
                            first = i_mm == 0
                            last = i_mm == n_mm2 - 1
                            nc.tensor.matmul(
                                psA[:64, :CS], zt(ls, KT_H + k),
                                t_wv[rs][:, k*VS + 2*CS*j:k*VS + 2*CS*j + CS],
                                start=first, stop=last, tile_position=(0, 0),
                                skip_group_check=True)
                            nc.tensor.matmul(
                                psB[64:128, :CS], zt(ls, KT_H + k),
                                t_wv[rs][:, k*VS + 2*CS*j + CS:
                                         k*VS + 2*CS*(j+1)],
                                start=first, stop=last, tile_position=(0, 64),
                                skip_group_check=True)
                            i_mm += 1
                    lg = wp.tile([128, CS], dt.float32, tag="lg", name=f"lg_{t}_{j}", uniquify=False)
                    nc.vector.tensor_tensor(lg[:64, :], psA[:64, :CS],
                                            std_t[:64, j*CS:(j+1)*CS],
                                            op=A.add)
                    nc.vector.tensor_tensor(lg[64:, :], psB[64:128, :CS],
                                            std_t[64:, j*CS:(j+1)*CS],
                                            op=A.add)
                    nc.vector.max(mx8[:, j*8:(j+1)*8], lg[:])
                    nc.vector.max_index(mi8[:, j*8:(j+1)*8],
                                        mx8[:, j*8:(j+1)*8], lg[:])

                # ---- F: pack candidates, AllGather, combine ----
                xchC = wp.tile([128, 8], dt.float32, tag="xchC")
                mxv = mx8[:].rearrange("p (j e) -> p j e", e=8)[:, :, 0]
                miv = mi8[:].rearrange("p (j e) -> p j e", e=8)[:, :, 0]
                candf = wp.tile([128, 4], dt.float32, tag="candf")
                nc.vector.tensor_copy(xchC[:, 0:4], mxv)
                nc.vector.tensor_copy(candf[:], miv)
                nc.vector.tensor_tensor(xchC[:, 4:8], candf[:], t_coff[:],
                                        op=A.add)
                nc.sync.dma_start(agc_in[par].ap(), xchC[:])
                if not NO_CC:
                    nc.gpsimd.collective_compute(
                        "AllGather", A.bypass, replica_groups=rg,
                        ins=[agc_in[par].ap()], outs=[agc_out[par].ap()])
                vals = wp.tile([64, 64], dt.float32, tag="vals", name=f"vals_{t}", uniquify=False)
                idxs = wp.tile([64, 64], dt.float32, tag="idxs", name=f"idxs_{t}", uniquify=False)
                srcc = agc_out[par].ap().rearrange(
                    "(co half b) x -> b co half x", co=C, half=2)
                nc.sync.dma_start(
                    vals[:].rearrange("b (co half j) -> b co half j",
                                      co=C, half=2), srcc[:, :, :, 0:4])
                nc.sync.dma_start(
                    idxs[:].rearrange("b (co half j) -> b co half j",
                                      co=C, half=2), srcc[:, :, :, 4:8])
                m1 = wp.tile([64, 1], dt.float32, tag="m1")
                nc.vector.reduce_max(m1[:], vals[:], axis=mybir.AxisListType.X)
                eqm = wp.tile([64, 64], dt.float32, tag="eqm")
                nc.vector.tensor_scalar(eqm[:], vals[:], m1[:, :1], None,
                                        op0=A.is_equal)
                nidx = wp.tile([64, 64], dt.float32, tag="nidx")
                nc.vector.tensor_scalar(nidx[:], idxs[:], -1.0, BIG,
                                        op0=A.mult, op1=A.add)
                key = wp.tile([64, 64], dt.float32, tag="key")
                nc.vector.tensor_tensor(key[:], eqm[:], nidx[:], op=A.mult)
                sel = wp.tile([64, 1], dt.float32, tag="sel")
                nc.vector.reduce_max(sel[:], key[:], axis=mybir.AxisListType.X)
                nc.vector.tensor_scalar(t_tokf[:], sel[:], -1.0, BIG,
                                        op0=A.mult, op1=A.add)
                nc.vector.tensor_copy(t_toku[:], t_tokf[:])
                nc.vector.tensor_copy(t_hist[:, t:t+1], t_tokf[:])

            # ---- final outputs ----
            nc.sync.dma_start(o_toks.ap(), t_hist[:])
            nc.sync.dma_start(o_h.ap(), t_h[:])

    nc.compile()
    return nc


def _split(x):
    hi = x.astype(bf16)
    lo = (x - hi.astype(np.float32)).astype(bf16)
    return hi, lo


def _prep_inputs(encoder_output, std, h, c, emb, w_ih, w_hh, b_ih, b_hh,
                 out_w, out_b):
    """Build the 8 per-core in_maps (all numpy, host side)."""
    emb = np.asarray(emb, np.float32)
    std = np.asarray(std, np.float32)
    h = np.asarray(h, np.float32)
    c = np.asarray(c, np.float32)
    w_ih = np.asarray(w_ih, np.float32)
    w_hh = np.asarray(w_hh, np.float32)
    bias = np.asarray(b_ih, np.float32) + np.asarray(b_hh, np.float32)  # [4,H]
    out_w = np.asarray(out_w, np.float32)
    out_b = np.asarray(out_b, np.float32)

    # z weights: [2048, 4, 1024] (K = [x;h], gate, H_out)
    Wz = np.concatenate([w_ih, w_hh], axis=1).transpose(1, 0, 2)
    ident = np.eye(128, dtype=np.float32)

    if SCHEME == "bf16x3":
        ehi, elo = _split(emb)
        emb_cat = np.concatenate([ehi, elo], axis=1)
        h0T_full = {k: v for k, v in zip(("hi", "lo"), _split(h.T.copy()))}
    else:
        emb_cat = emb
        h0T_full = {"r": h.T.copy()}

    in_maps = []
    for cc in range(C):
        m = {"emb_cat": emb_cat}
        # gate columns permuted to [sub(2), gate(4), j(64)] for col-tiling
        wzc = Wz[:, :, HS*cc:HS*(cc+1)]                  # [2048, 4, 128]
        wzc = wzc.reshape(2048, 4, 2, 64).transpose(0, 2, 1, 3) \
                 .reshape(2048, 512)
        wvc = out_w[:, VS*cc:VS*(cc+1)]                  # [1024, 4000]
        if SCHEME == "bf16x3":
            for s, v in zip(("hi", "lo"), _split(wzc)):
                m[f"wz_{s}"] = v.reshape(KT_Z, 128, 512)
            for s, v in zip(("hi", "lo"), _split(wvc)):
                m[f"wv_{s}"] = v.reshape(KT_H, 128, VS)
            for s in ("hi", "lo"):
                m[f"h0T_{s}"] = h0T_full[s].reshape(KT_H, 128, 64)
        else:
            m["wz_r"] = wzc.reshape(KT_Z, 128, 512)
            m["wv_r"] = wvc.reshape(KT_H, 128, VS)
            m["h0T_r"] = h0T_full["r"].reshape(KT_H, 128, 64)
        bc = bias[:, HS*cc:HS*(cc+1)].reshape(4, 2, 64).transpose(1, 0, 2) \
                 .reshape(2, 256)                        # [sub, gate*64]
        m["bias"] = np.repeat(bc, 64, axis=0).astype(np.float32)  # [128,256]
        m["c0p"] = np.ascontiguousarray(
            c[:, HS*cc:HS*(cc+1)].reshape(64, 2, 64).transpose(1, 0, 2)
            .reshape(128, 64), np.float32)
        p = np.arange(128)
        j = np.arange(4)
        m["cand_off"] = (VS*cc + 2*CS*j[None, :]
                         + CS*(p[:, None] // 64)).astype(np.float32)
        # std: [T, 64, 4000] -> [T, 128, 2048] with col j*512+q ->
        # vocab VS*cc + 1024*j + 512*sub + q, plus out_b folded in
        sc = std[:T, :, VS*cc:VS*(cc+1)] + out_b[None, None, VS*cc:VS*(cc+1)]
        sc = sc.reshape(T, 64, NPAIR, 2, CS).transpose(0, 3, 1, 2, 4) \
               .reshape(T, 128, NPAIR * CS)
        m["stdp"] = np.ascontiguousarray(sc, np.float32)
        m["ident"] = ident
        m["identb"] = ident.astype(bf16)
        in_maps.append(m)
    return in_maps


def _run_pjrt(nc, in_maps, time_iters=0):
    """Mirror of bass2jax.run_bass_via_pjrt with pre-placed device inputs and
    an optional timing loop (wall time of execute with inputs resident)."""
    import concourse.mybir as mb
    _b2j.install_neuronx_cc_hook()
    if nc.dbg_addr is not None:
        in_maps = [{**m, nc.dbg_addr.name: np.zeros((1, 2), np.uint32)}
                   for m in in_maps]
    partition_name = (nc.partition_id_tensor.name
                      if nc.partition_id_tensor else None)
    in_names, out_names, out_avals, zero_outs = [], [], [], []
    for alloc in nc.m.functions[0].allocations:
        if not isinstance(alloc, mb.MemoryLocationSet):
            continue
        name = alloc.memorylocations[0].name
        if alloc.kind == "ExternalInput":
            if name != partition_name:
                in_names.append(name)
        elif alloc.kind == "ExternalOutput":
            out_names.append(name)
            shape = tuple(alloc.tensor_shape)
            dtp = mb.dt.np(alloc.dtype)
            out_avals.append(jax.core.ShapedArray(shape, dtp))
            zero_outs.append(np.zeros(shape, dtp))
    n_params = len(in_names)
    n_outs = len(out_avals)
    in_names.extend(out_names)
    if partition_name is not None:
        in_names.append(partition_name)

    def _body(*args):
        operands = list(args)
        if partition_name is not None:
            operands.append(_b2j.partition_id_tensor())
        outs = _b2j._bass_exec_p.bind(
            *operands, out_avals=tuple(out_avals), in_names=tuple(in_names),
            out_names=tuple(out_names), lowering_input_output_aliases=(),
            sim_require_finite=True, sim_require_nnan=True, nc=nc)
        return tuple(outs)

    devices = jax.devices()[:C]
    mesh = Mesh(np.asarray(devices), ("core",))
    in_specs = (PartitionSpec("core"),) * (n_params + n_outs)
    out_specs = (PartitionSpec("core"),) * len(out_names)
    sharded = jax.jit(shard_map(_body, mesh=mesh, in_specs=in_specs,
                                out_specs=out_specs, check_rep=False),
                      keep_unused=True)
    sh = NamedSharding(mesh, PartitionSpec("core"))
    concat_in = [
        jax.device_put(np.concatenate(
            [np.asarray(in_maps[c][in_names[i]]) for c in range(C)], axis=0),
            sh)
        for i in range(n_params)
    ]
    concat_zeros = [
        jax.device_put(np.zeros((C * z.shape[0], *z.shape[1:]), z.dtype), sh)
        for z in zero_outs
    ]
    t0 = time.time()
    out_arrs = jax.block_until_ready(sharded(*concat_in, *concat_zeros))
    print(f"[kernel] first exec (incl compile): {time.time()-t0:.1f}s",
          file=sys.stderr)
    exec_times = []
    for _ in range(time_iters):
        t0 = time.time()
        out_arrs2 = jax.block_until_ready(sharded(*concat_in, *concat_zeros))
        exec_times.append(time.time() - t0)
        del out_arrs2
    results = [
        {name: np.asarray(out_arrs[i]).reshape(C, *out_avals[i].shape)[c]
         for i, name in enumerate(out_names)}
        for c in range(C)
    ]
    return results, (min(exec_times) if exec_times else None)


def kernel(**inputs):
    if "nc" not in _cache:
        t0 = time.time()
        _cache["nc"] = build()
        print(f"[kernel] build: {time.time()-t0:.1f}s", file=sys.stderr)
    nc = _cache["nc"]
    in_maps = _prep_inputs(**inputs)
    t0 = time.time()
    results, best_s = _run_pjrt(nc, in_maps,
                                time_iters=int(os.environ.get("KERNEL_TIME_ITERS", "0")))
    print(f"[kernel] run: {time.time()-t0:.1f}s", file=sys.stderr)
    _cache["exec_wall_s"] = best_s

    toks = results[0]["toks_out"].T.astype(np.int32)  # [T, 64]
    h_full = np.zeros((64, H), np.float32)
    for cc in range(C):
        hv = results[cc]["h_out"].reshape(2, 64, 64)     # [sub, b, j]
        h_full[:, HS*cc:HS*cc+64] = hv[0]
        h_full[:, HS*cc+64:HS*(cc+1)] = hv[1]
    return toks, h_full
